# revision 12
# baseline (speedup 1.0000x reference)
"""Causal self-attention Trainium2 Bass kernel.

Problem: B=128, T=256, D=512, H=8 heads of 64. f32 in/out.
Sharding: data-parallel over batch — 16 batches per NeuronCore, weights
replicated, no collectives.

Matmul datapath in fp16 (e5m10): 4-byte fp32r streams cap at ~1.2GHz
on the PE moving-operand port; 2-byte streams run the full 2.4GHz and
get FWL fast weight loads. PSUM accumulation stays fp32.

Per batch-pair (b0, b1):
  1. xT fed pre-transposed from host as [bl, 128, 4, 256] (d-major).
  2. QK projection feature-major over BOTH batches at once (rhs N=512
     halves the weight-load count); Q columns of W pre-scaled by
     1/sqrt(hd) on host; per-partition bias added during PSUM evac.
  3. Per batch: V projection token-major; V bias folded into a host
     effective output bias (rows of softmax sum to 1).
  4. Per head: S^T[s,t] both s-tiles into ONE [128,384] PSUM bank
     (s1 only needs t in [128,256) by causality), single exp ->
     E^T [128,384] in SBUF, multiplicative causal mask on the two
     diagonal blocks only (one strided DVE op).
  5. O[t,hd] = matmul(lhsT=E^T slice, rhs=[V_h | 1 | pad]); col 64 is
     the softmax denominator per token (per-partition). Both t-tiles
     share one PSUM bank; one reciprocal + one broadcast-multiply
     normalizes and evacuates into a head-pair staging tile.
  6. Head-pair [128,128] fp32r PE transposes to feature-major OT;
     y = OT.T @ W_out + b_eff.
"""

import numpy as np

B, T, D = 128, 256, 512
H, HD = 8, 64
NCORES = 8
BL = B // NCORES  # batches per core


def build_nc(bl=BL, num_devices=NCORES):
    from contextlib import ExitStack

    import concourse.bacc as bacc
    import concourse.tile as tile
    from concourse import mybir

    f32 = mybir.dt.float32
    f16 = mybir.dt.float16
    AF = mybir.ActivationFunctionType

    nc = bacc.Bacc(
        "TRN2",
        target_bir_lowering=False,
        debug=False,
        enable_asserts=False,
        num_devices=num_devices,
    )

    xt_d = nc.dram_tensor("xt", [bl, 128, 4, 256], f16, kind="ExternalInput").ap()
    w_d = nc.dram_tensor("wqkv", [D, 3 * D], f16, kind="ExternalInput").ap()
    wo_d = nc.dram_tensor("wout", [D, D], f16, kind="ExternalInput").ap()
    bqk_d = nc.dram_tensor("bqk", [128, 8], f32, kind="ExternalInput").ap()
    beff_d = nc.dram_tensor("beff", [128, D], f32, kind="ExternalInput").ap()
    bm_d = nc.dram_tensor("binm", [128, 128], f16, kind="ExternalInput").ap()
    id_d = nc.dram_tensor("ident", [128, 128], f16, kind="ExternalInput").ap()
    y_d = nc.dram_tensor("y", [bl, T, D], f32, kind="ExternalOutput").ap()

    with tile.TileContext(nc) as tc, ExitStack() as ctx:
        singles = ctx.enter_context(tc.tile_pool(name="singles", bufs=1))
        p_xt = ctx.enter_context(tc.tile_pool(name="p_xt", bufs=2))
        p_qkt = ctx.enter_context(tc.tile_pool(name="p_qkt", bufs=2))
        p_va = ctx.enter_context(tc.tile_pool(name="p_va", bufs=2))
        p_et = ctx.enter_context(tc.tile_pool(name="p_et", bufs=3))
        p_o = ctx.enter_context(tc.tile_pool(name="p_o", bufs=3))
        p_li = ctx.enter_context(tc.tile_pool(name="p_li", bufs=4))
        p_ot = ctx.enter_context(tc.tile_pool(name="p_ot", bufs=2))
        p_y = ctx.enter_context(tc.tile_pool(name="p_y", bufs=3))
        psA = ctx.enter_context(tc.tile_pool(name="psA", bufs=3, space="PSUM"))
        psB = ctx.enter_context(tc.tile_pool(name="psB", bufs=3, space="PSUM"))
        psC = ctx.enter_context(tc.tile_pool(name="psC", bufs=2, space="PSUM"))

        w_sb = singles.tile([128, 4, 3 * D], f16, tag="w")
        nc.sync.dma_start(out=w_sb, in_=w_d.rearrange("(k p) n -> p k n", p=128))
        wo_sb = singles.tile([128, 4, D], f16, tag="wo")
        nc.sync.dma_start(out=wo_sb, in_=wo_d.rearrange("(k p) n -> p k n", p=128))
        bqk_sb = singles.tile([128, 8], f32, tag="bqk")
        nc.sync.dma_start(out=bqk_sb, in_=bqk_d)
        beff_sb = singles.tile([128, D], f32, tag="beff")
        nc.sync.dma_start(out=beff_sb, in_=beff_d)
        bm_sb = singles.tile([128, 128], f16, tag="bm")
        nc.sync.dma_start(out=bm_sb, in_=bm_d)
        id_sb = singles.tile([128, 128], f16, tag="id")
        nc.sync.dma_start(out=id_sb, in_=id_d)

        for pair in range(bl // 2):
            # ---- load xT for both batches of the pair ----
            xt = p_xt.tile([128, 4, 512], f16, tag="xt")
            for bb in range(2):
                nc.sync.dma_start(
                    out=xt[:, :, bb * 256 : (bb + 1) * 256],
                    in_=xt_d[pair * 2 + bb],
                )

            # ---- QK projection, feature-major, both batches (N=512) ----
            qkt = p_qkt.tile([128, 8, 512], f16, tag="qkt")
            for f in range(8):
                qp = psB.tile([128, 512], f32, tag="s")
                for k in range(4):
                    nc.tensor.matmul(
                        qp,
                        lhsT=w_sb[:, k, f * 128 : (f + 1) * 128],
                        rhs=xt[:, k, :],
                        start=(k == 0),
                        stop=(k == 3),
                    )
                if f % 2 == 0:
                    nc.scalar.add(qkt[:, f, :], qp, bqk_sb[:, f : f + 1])
                else:
                    nc.vector.tensor_scalar_add(
                        qkt[:, f, :], qp, bqk_sb[:, f : f + 1]
                    )

            for bb in range(2):
                b = pair * 2 + bb
                tb = bb * 256  # this batch's token offset inside pair tiles

                # ---- V projection, token-major, ones column per head ----
                va = p_va.tile([128, 2, 8, 66], f16, tag="va")
                for st in range(2):
                    vp = psC.tile([128, 512], f32, tag="v")
                    for k in range(4):
                        nc.tensor.matmul(
                            vp,
                            lhsT=xt[:, k, tb + st * 128 : tb + (st + 1) * 128],
                            rhs=w_sb[:, k, 2 * D : 3 * D],
                            start=(k == 0),
                            stop=(k == 3),
                        )
                    nc.any.tensor_copy(
                        out=va[:, st, :, 0:64],
                        in_=vp.rearrange("p (h c) -> p h c", c=64),
                    )
                    nc.vector.memset(
                        va[:, st, :, 64:66].bitcast(mybir.dt.uint32), 0x3C003C00
                    )

                # ---- attention, head pairs ----
                otsb = p_ot.tile([128, 4, 256], f16, tag="ot")
                for fp in range(4):
                    osb = p_o.tile([128, 2, 128], f16, tag="o")
                    # issue both heads' S matmuls adjacent: K=64 at row
                    # groups 0-63 / 64-127 -> they run concurrently on PE
                    sps = []
                    for st in range(2):
                        for hh in range(2):
                            po = hh * 64
                            qt = qkt[po : po + 64, fp, tb : tb + 256]
                            kt = qkt[po : po + 64, 4 + fp, tb : tb + 256]
                            if st == 0:
                                sp = psB.tile([128, 384], f32, tag="s")
                                sps.append(sp)
                                nc.tensor.matmul(
                                    sp[:, 0:256], lhsT=kt[:, 0:128], rhs=qt,
                                    start=True, stop=True,
                                )
                            else:
                                nc.tensor.matmul(
                                    sps[hh][:, 256:384], lhsT=kt[:, 128:256],
                                    rhs=qt[:, 128:256], start=True, stop=True,
                                )
                    ets = []
                    for hh in range(2):
                        et = p_et.tile([128, 384], f16, tag="et")
                        ets.append(et)
                        nc.scalar.activation(et, sps[hh], AF.Exp)
                        # multiplicative causal mask on the two diagonal
                        # blocks (cols 0:128 = s0/t0, 256:384 = s1/t1)
                        dv = et.rearrange("p (a c) -> p a c", a=3)[:, 0::2, :]
                        nc.vector.tensor_mul(
                            out=dv, in0=dv,
                            in1=bm_sb[:, None, :].broadcast_to([128, 2, 128]),
                        )
                    for hh in range(2):
                        h = 2 * fp + hh
                        po = hh * 64
                        et = ets[hh]
                        # O matmuls: both t-tiles share one PSUM bank
                        op = psA.tile([128, 2, 66], f32, tag="a")
                        nc.tensor.matmul(
                            op[:, 0, :], lhsT=et[:, 0:128], rhs=va[:, 0, h, :],
                            start=True, stop=True,
                        )
                        nc.tensor.matmul(
                            op[:, 1, :], lhsT=et[:, 128:256], rhs=va[:, 0, h, :],
                            start=True, stop=False,
                        )
                        nc.tensor.matmul(
                            op[:, 1, :], lhsT=et[:, 256:384], rhs=va[:, 1, h, :],
                            start=False, stop=True,
                        )
                        li = p_li.tile([128, 2], f32, tag="li")
                        nc.vector.reciprocal(li, op[:, :, 64])
                        nc.vector.tensor_mul(
                            out=osb[:, :, po : po + 64],
                            in0=op[:, :, 0:64],
                            in1=li[:, :, None].broadcast_to([128, 2, 64]),
                        )
                    # head-pair transposes to feature-major
                    for tt in range(2):
                        otp = psA.tile([128, 128], f16, tag="a")
                        nc.tensor.transpose(otp, osb[:, tt, :], id_sb)
                        nc.any.tensor_copy(
                            out=otsb[:, fp, tt * 128 : (tt + 1) * 128], in_=otp
                        )

                # ---- output projection ----
                for tt in range(2):
                    yp = psC.tile([128, 512], f32, tag="v")
                    for f in range(4):
                        nc.tensor.matmul(
                            yp,
                            lhsT=otsb[:, f, tt * 128 : (tt + 1) * 128],
                            rhs=wo_sb[:, f, :],
                            start=(f == 0),
                            stop=(f == 3),
                        )
                    ysb = p_y.tile([128, 512], f32, tag="y")
                    nc.vector.tensor_add(out=ysb, in0=yp, in1=beff_sb)
                    nc.sync.dma_start(
                        out=y_d[b, tt * 128 : (tt + 1) * 128, :], in_=ysb
                    )

    nc.compile()
    return nc


def host_inputs(x, W_qkv, b_qkv, W_out, b_out):
    """Host-side preprocessing. Returns per-core-shared inputs plus the
    transposed x layout [B, 128, 4, 256] (d-major tiles)."""
    scale = 1.0 / np.sqrt(HD)
    W = np.array(W_qkv, dtype=np.float32).copy()
    W[:, :D] *= scale  # fold attention scale into Q projection
    bq = np.array(b_qkv, dtype=np.float64).copy()
    bq[:D] *= scale
    bqk = np.stack([bq[j * 128 : (j + 1) * 128] for j in range(8)], axis=1).astype(
        np.float32
    )
    beff_row = (
        np.array(b_qkv[2 * D :], np.float64) @ np.array(W_out, np.float64)
        + np.array(b_out, np.float64)
    ).astype(np.float32)
    beff = np.broadcast_to(beff_row, (128, D)).copy()
    i = np.arange(128)[:, None]
    j = np.arange(128)[None, :]
    binm = (j >= i).astype(np.float32)  # 1 on/above diagonal (t >= s)
    ident = np.eye(128, dtype=np.float32)
    return {
        "wqkv": W.astype(np.float16),
        "wout": np.array(W_out, np.float16),
        "bqk": bqk,
        "beff": beff,
        "binm": binm.astype(np.float16),
        "ident": ident.astype(np.float16),
    }


def xt_layout(x):
    """[B, T, D] -> [B, 128, 4, 256]: xt[b, p, k, t] = x[b, t, 128k+p]."""
    xb = np.asarray(x, dtype=np.float32)
    return np.ascontiguousarray(
        xb.transpose(0, 2, 1).reshape(-1, 4, 128, T).transpose(0, 2, 1, 3)
    ).astype(np.float16)


def kernel(x, W_qkv, b_qkv, W_out, b_out):
    from concourse.bass_utils import run_bass_kernel_spmd

    shared = host_inputs(x, W_qkv, b_qkv, W_out, b_out)
    xt = xt_layout(x)
    nc = build_nc(BL, NCORES)
    in_maps = [
        {"xt": xt[c * BL : (c + 1) * BL], **shared} for c in range(NCORES)
    ]
    res = run_bass_kernel_spmd(nc, in_maps, core_ids=list(range(NCORES)))
    y = np.concatenate([res.results[c]["y"] for c in range(NCORES)], axis=0)
    return y.astype(np.float32)
